# revision 4
# baseline (speedup 1.0000x reference)
"""Trainium2 Bass kernel for nn_AdaptiveAggregationLayer (GNN message passing).

Design (977 us measured vs 1218 us baseline):
  - Destinations sharded over 8 cores (12500 nodes each), x replicated;
    edges bucketed by (dest window of 128, source region of 25000).
  - The binding constraint is gpsimd dma_gather descriptor generation
    (~2.4 ns per gathered row, byte-size independent), so sources are
    DEDUPLICATED per bucket (one gather per unique source; the one-hot
    aggregation matrix S carries src->dst multiplicities) and gathers read
    exactly maxcnt16 rows per bucket (tail rows of the last block are
    memset so S=0 never multiplies stale fp8 NaNs).
  - Edge stream in fp8e4m3; segment-sum via PE matmuls with fp8 DoubleRow
    perf mode (pairs of 128-edge blocks per instruction); per-window S
    tiles streamed fp8 from DRAM.
  - Epilogue per 128-node window: mean = nbsum * invdeg (ACT), x_own
    shipped pre-transposed, mean transposed on PE, h_mean/h_concat as one
    PSUM-accumulated matmul chain against stacked bf16 weights, sigmoid
    gate mix, bf16 writeback (host converts to f32).
"""
import math
import os
import numpy as np

import concourse.bass as bass
import concourse.bacc as bacc
import concourse.mybir as mybir
from concourse import tile
from concourse.bass_utils import run_bass_kernel_spmd

F32 = mybir.dt.float32
BF16 = mybir.dt.bfloat16
FP8 = mybir.dt.float8e4
I16 = mybir.dt.int16

CFG = dict(
    N=100000,
    F=256,
    CORES=8,
    REG=4,   # source regions (int16 gather idx < 32768)
    KW=int(os.environ.get("KW", "4")),  # windows per superwindow
)

LAST_EXEC_NS = None
LAST_RESULTS = None


def _derive(cfg):
    N, CORES, KW = cfg["N"], cfg["CORES"], cfg["KW"]
    NPC = N // CORES
    NWIN = math.ceil(NPC / 128)
    NPCP = NWIN * 128
    REGSZ = math.ceil(N / cfg["REG"])
    assert REGSZ < 32768
    NSW = math.ceil(NWIN / KW)
    return NPC, NWIN, NPCP, REGSZ, NSW


def _host_prep(x, edge_index, delta_agg, cfg):
    """Sort/pad edges into (superwindow, region, window) block order.

    Returns per-core arrays + the shared shape tables.
    """
    N, F, CORES, REG, KW = cfg["N"], cfg["F"], cfg["CORES"], cfg["REG"], cfg["KW"]
    NPC, NWIN, NPCP, REGSZ, NSW = _derive(cfg)

    row = np.asarray(edge_index[0]).astype(np.int64)
    col = np.asarray(edge_index[1]).astype(np.int64)

    c = row // NPC
    loc = row - c * NPC
    w = loc >> 7
    d = loc & 127
    b = col // REGSZ
    lcol = (col - b * REGSZ).astype(np.int16)
    sw = w // KW

    # Sort edges by (core, sw, region, window, source) and deduplicate
    # sources within each (core, window, region) bucket: each unique source
    # is gathered once; S carries the (src -> dst) multiplicity.
    key = ((c * NSW + sw) * REG + b) * NWIN + w
    order = np.lexsort((col, key))
    k_s = key[order]
    col_s = col[order]
    c_s, w_s, b_s, d_s, lcol_s = (
        c[order], w[order], b[order], d[order], lcol[order],
    )
    newseg = np.r_[True, k_s[1:] != k_s[:-1]]
    newsrc = newseg | np.r_[True, col_s[1:] != col_s[:-1]]
    gid = np.cumsum(newsrc) - 1
    seg_ids = np.cumsum(newseg) - 1
    seg_first_gid = gid[np.flatnonzero(newseg)]
    pos = gid - seg_first_gid[seg_ids]  # unique-source slot within bucket

    # unique counts per (core, window, region) bucket
    ucounts = np.zeros((CORES, NWIN, REG), dtype=np.int64)
    usrc_mask = newsrc
    np.add.at(
        ucounts,
        (c_s[usrc_mask], w_s[usrc_mask], b_s[usrc_mask]),
        1,
    )
    maxcnt = ucounts.max(axis=0)               # [NWIN, REG]
    maxcnt16 = np.maximum(maxcnt, 0)           # exact per-bucket rows
    nblk = (maxcnt16 + 127) // 128             # [NWIN, REG] shared
    empty_w = nblk.sum(axis=1) == 0
    nblk[empty_w, 0] = 1
    maxcnt16 = np.maximum(maxcnt16, 16 * (nblk > 0))

    # Global block order: for sw: for b: for w in sw.
    blk0 = np.zeros((NWIN, REG), dtype=np.int64)
    nbg = np.zeros((NSW, REG), dtype=np.int64)   # blocks per gather group
    gb0 = np.zeros((NSW, REG), dtype=np.int64)   # group start block
    W0sw = np.zeros(NSW, dtype=np.int64)
    Tsw = np.zeros(NSW, dtype=np.int64)
    off = 0
    for s in range(NSW):
        W0sw[s] = off
        wlo, whi = s * KW, min((s + 1) * KW, NWIN)
        for bi in range(REG):
            gb0[s, bi] = off
            for wi in range(wlo, whi):
                blk0[wi, bi] = off
                off += nblk[wi, bi]
            nbg[s, bi] = off - gb0[s, bi]
        Tsw[s] = off - W0sw[s]
    TOTBLK = off

    slot = blk0[w_s, b_s] * 128 + pos  # per-core global slot (deduped)

    deg = np.bincount(row, minlength=N).astype(np.float32)
    invdeg = 1.0 / np.maximum(deg, 1.0)
    delta = np.asarray(delta_agg).astype(np.float32)

    x_np = np.asarray(x, dtype=np.float32)
    fp8np = mybir.dt.np(FP8)
    bf16np = mybir.dt.np(BF16)
    x_fp8 = x_np.astype(fp8np)

    core_edge_ends = np.cumsum(np.bincount(c_s, minlength=CORES))
    core_edge_starts = core_edge_ends - np.bincount(c_s, minlength=CORES)

    per_core = []
    for ci in range(CORES):
        lo, hi = core_edge_starts[ci], core_edge_ends[ci]
        sl = slot[lo:hi]
        flat_idx = np.zeros(TOTBLK * 128, np.int16)
        flat_idx[sl] = lcol_s[lo:hi]
        Sf = np.zeros((128, TOTBLK * 128), dtype=np.float32)
        np.add.at(Sf, (sl % 128, (sl // 128) * 128 + d_s[lo:hi]), 1.0)
        S = Sf.astype(fp8np)
        src_idx = np.tile(
            np.ascontiguousarray(flat_idx.reshape(TOTBLK * 8, 16).T), (8, 1)
        )

        xq = x_np[ci * NPC : (ci + 1) * NPC]  # [NPC, F]
        xT = np.zeros((128, 2, NPCP), bf16np)
        xT[:, 0, :NPC] = xq[:, 0:128].T.astype(bf16np)
        xT[:, 1, :NPC] = xq[:, 128:F].T.astype(bf16np)

        ivc = np.zeros(NPCP, np.float32)
        ivc[:NPC] = invdeg[ci * NPC : (ci + 1) * NPC]
        dlc = np.zeros(NPCP, np.float32)
        dlc[:NPC] = delta[ci * NPC : (ci + 1) * NPC]
        per_core.append(
            dict(
                src_idx=src_idx,
                S=S,
                xT=xT,
                invdeg=ivc.reshape(NWIN, 128).T.copy(),
                delta=dlc.reshape(NWIN, 128).T.copy(),
            )
        )

    shape = dict(
        nblk=nblk, blk0=blk0, nbg=nbg, gb0=gb0, W0sw=W0sw, Tsw=Tsw,
        TOTBLK=TOTBLK, maxcnt16=maxcnt16,
    )
    return per_core, shape, x_fp8


def _build_graph(cfg, shape, gate_weight, gate_bias):
    N, F, REG, KW = cfg["N"], cfg["F"], cfg["REG"], cfg["KW"]
    NPC, NWIN, NPCP, REGSZ, NSW = _derive(cfg)
    nblk, blk0, nbg, gb0, W0sw, Tsw, TOTBLK = (
        shape["nblk"], shape["blk0"], shape["nbg"], shape["gb0"],
        shape["W0sw"], shape["Tsw"], shape["TOTBLK"],
    )
    maxcnt16 = shape["maxcnt16"]

    nc = bacc.Bacc("TRN2", target_bir_lowering=False, debug=False,
                   num_swdge_queues=4)

    x_d = nc.dram_tensor("x", [N, F], FP8, kind="ExternalInput")
    xT_d = nc.dram_tensor("xT", [128, 2, NPCP], BF16, kind="ExternalInput")
    srcidx_d = nc.dram_tensor("src_idx", [128, TOTBLK * 8], I16, kind="ExternalInput")
    s_d = nc.dram_tensor("S", [128, TOTBLK * 128], FP8, kind="ExternalInput")
    invd_d = nc.dram_tensor("invdeg", [128, NWIN], F32, kind="ExternalInput")
    delt_d = nc.dram_tensor("delta", [128, NWIN], F32, kind="ExternalInput")
    wc_d = nc.dram_tensor("WC", [512, 2 * F], BF16, kind="ExternalInput")
    bc2_d = nc.dram_tensor("bC", [1, 2 * F], BF16, kind="ExternalInput")
    idn_d = nc.dram_tensor("ident", [128, 128], BF16, kind="ExternalInput")
    ones_d = nc.dram_tensor("ones", [1, 128], BF16, kind="ExternalInput")
    out_d = nc.dram_tensor("out", [NPCP, F], BF16, kind="ExternalOutput")

    AT = mybir.ActivationFunctionType
    OP = mybir.AluOpType

    with tile.TileContext(nc) as tc:
        with (
            tc.tile_pool(name="const", bufs=1) as cpool,
            tc.tile_pool(name="main", bufs=2) as pool,
            tc.tile_pool(name="gstream", bufs=3) as spool,
            tc.tile_pool(name="sstream", bufs=6) as spoolS,
            tc.tile_pool(name="psum", bufs=2, space="PSUM") as ppool,
            tc.tile_pool(name="psum3", bufs=3, space="PSUM") as ppool3,
        ):
            wc = cpool.tile([128, 4, 2 * F], BF16, tag="wc")
            for k in range(4):
                nc.scalar.dma_start(out=wc[:, k, :], in_=wc_d[k * 128 : (k + 1) * 128, :])
            bc2 = cpool.tile([1, 2 * F], BF16, tag="bc2")
            nc.scalar.dma_start(out=bc2[:, :], in_=bc2_d[:, :])
            ones = cpool.tile([1, 128], BF16, tag="ones")
            nc.scalar.dma_start(out=ones[:, :], in_=ones_d[:, :])
            idn = cpool.tile([128, 128], BF16, tag="idn")
            nc.scalar.dma_start(out=idn[:, :], in_=idn_d[:, :])
            invd = cpool.tile([128, NWIN], F32, tag="invd")
            nc.scalar.dma_start(out=invd[:, :], in_=invd_d[:, :])
            delt = cpool.tile([128, NWIN], F32, tag="delt")
            nc.scalar.dma_start(out=delt[:, :], in_=delt_d[:, :])

            g = cpool.tile([128, NWIN], F32, tag="g")
            nc.scalar.activation(
                g[:, :], delt[:, :], AT.Sigmoid,
                bias=float(gate_bias), scale=float(gate_weight),
            )
            omg = cpool.tile([128, NWIN], F32, tag="omg")
            nc.vector.tensor_scalar(omg[:, :], g[:, :], -1.0, 1.0, OP.mult, OP.add)

            gq = [0]
            for s in range(NSW):
                T = int(Tsw[s])
                w0 = int(W0sw[s])
                wlo, whi = s * KW, min((s + 1) * KW, NWIN)

                idxw = spool.tile([128, T * 8], I16, tag="idxw")
                nc.sync.dma_start(
                    out=idxw[:, :], in_=srcidx_d[:, w0 * 8 : (w0 + T) * 8]
                )
                gath = spool.tile([128, T, F], FP8, tag="gath")
                swins = {}
                # Per-window S loads + per-bucket gathers, window-major so the
                # first window's dependencies complete earliest.
                for w in range(wlo, whi):
                    Twin = int(nblk[w].sum())
                    swin = spoolS.tile([128, Twin, 128], FP8, tag="swin")
                    swins[w] = swin
                    so = 0
                    for bi in range(REG):
                        nb = int(nblk[w, bi])
                        if nb == 0:
                            continue
                        o = int(blk0[w, bi])
                        nc.scalar.dma_start(
                            out=swin[:, so : so + nb, :],
                            in_=s_d[:, o * 128 : (o + nb) * 128],
                        )
                        so += nb
                    for bi in range(REG):
                        nb = int(nblk[w, bi])
                        if nb == 0:
                            continue
                        o = int(blk0[w, bi]) - w0
                        ni = int(maxcnt16[w, bi])
                        if ni < nb * 128:
                            # gather skips the tail rows of the last block;
                            # zero them so S=0 never multiplies stale NaN fp8
                            nc.vector.memset(gath[:, o + nb - 1, :], 0.0)
                        nc.gpsimd.dma_gather(
                            gath[:, o : o + nb, :],
                            x_d[bi * REGSZ : min((bi + 1) * REGSZ, N), :],
                            idxw[:, o * 8 : o * 8 + (ni + 15) // 16],
                            ni,
                            ni,
                            F,
                            single_packet=False,
                            queue_num=gq[0] % 4,
                        )
                        gq[0] += 1

                for w in range(wlo, whi):
                    # matmul runs for window w: one per region.
                    # (gath offset within SW tile, swin offset, length)
                    runs = []
                    so = 0
                    for bi in range(REG):
                        nb = int(nblk[w, bi])
                        if nb:
                            runs.append((int(blk0[w, bi]) - w0, so, nb))
                            so += nb
                    swin = swins[w]
                    nbs = ppool3.tile([128, F], F32, tag="nbsum")
                    total = sum(nb for _, _, nb in runs)
                    done = 0
                    for o, so_r, nb in runs:
                        j = 0
                        while j < nb:
                            pair = 2 if (j + 1 < nb) else 1
                            if pair == 2:
                                nc.tensor.matmul(
                                    nbs[:, :],
                                    swin[:, so_r + j : so_r + j + 2, :],
                                    gath[:, o + j : o + j + 2, :],
                                    start=(done == 0),
                                    stop=(done + 2 == total),
                                    perf_mode=mybir.MatmulPerfMode.DoubleRow,
                                )
                            else:
                                nc.tensor.matmul(
                                    nbs[:, :],
                                    swin[:, so_r + j, :],
                                    gath[:, o + j, :],
                                    start=(done == 0),
                                    stop=(done + 1 == total),
                                )
                            j += pair
                            done += pair

                    mean = pool.tile([128, F], BF16, tag="mean")
                    nc.scalar.activation(
                        mean[:, :], nbs[:, :], AT.Copy, scale=invd[:, w : w + 1]
                    )
                    lhs = pool.tile([128, 512], BF16, tag="lhs")
                    nc.sync.dma_start(
                        out=lhs[:, 0:256], in_=xT_d[:, :, w * 128 : (w + 1) * 128]
                    )
                    tp = ppool.tile([128, 256], BF16, tag="tps")
                    nc.tensor.transpose(tp[:, 0:128], mean[:, 0:128], idn[:, :])
                    nc.tensor.transpose(tp[:, 128:256], mean[:, 128:256], idn[:, :])
                    nc.vector.tensor_copy(lhs[:, 256:512], tp[:, :])

                    hcomb = ppool.tile([128, 2 * F], F32, tag="hcomb")
                    nc.tensor.matmul(
                        hcomb[:, :], ones[:, :], bc2[:, :], start=True, stop=False,
                    )
                    for k in range(4):
                        nc.tensor.matmul(
                            hcomb[:, :],
                            lhs[:, k * 128 : (k + 1) * 128],
                            wc[:, k, :],
                            start=False,
                            stop=(k == 3),
                        )
                    av = pool.tile([128, F], F32, tag="av")
                    nc.scalar.activation(
                        av[:, :], hcomb[:, 0:F], AT.Copy, scale=omg[:, w : w + 1]
                    )
                    bv = pool.tile([128, F], F32, tag="bv")
                    nc.vector.tensor_scalar(
                        bv[:, :], hcomb[:, F : 2 * F], g[:, w : w + 1], None, OP.mult
                    )
                    ot = pool.tile([128, F], BF16, tag="ot")
                    nc.vector.tensor_tensor(ot[:, :], av[:, :], bv[:, :], op=OP.add)
                    nc.sync.dma_start(
                        out=out_d[w * 128 : (w + 1) * 128, :], in_=ot[:, :]
                    )
    nc.compile()
    return nc


def _make_weight_arrays(W_mean, b_mean, W_ego, b_ego, W_nb, b_nb, cfg):
    F = cfg["F"]
    EGO = W_ego.shape[1]
    W_mean = np.asarray(W_mean, np.float32)
    WA = np.concatenate([0.5 * W_mean, 0.5 * W_mean], axis=0)
    WB = np.zeros((2 * F, F), np.float32)
    WB[0:F, 0:EGO] = np.asarray(W_ego, np.float32)
    WB[F : 2 * F, EGO:F] = np.asarray(W_nb, np.float32)
    bm = np.asarray(b_mean, np.float32)[None, :]
    bcat = np.concatenate(
        [np.asarray(b_ego, np.float32), np.asarray(b_nb, np.float32)]
    )[None, :]
    WC = np.concatenate([WA, WB], axis=1)          # [512, 512]
    bC = np.concatenate([bm, bcat], axis=1)        # [1, 512]
    npdt = mybir.dt.np(BF16)
    idn = np.eye(128).astype(npdt)
    ones = np.ones((1, 128)).astype(npdt)
    return WC.astype(npdt), bC.astype(npdt), idn, ones


def run(inputs, cfg=None, trace=True, sim=False):
    global LAST_EXEC_NS, LAST_RESULTS
    cfg = dict(CFG if cfg is None else cfg)
    N, F, CORES = cfg["N"], cfg["F"], cfg["CORES"]
    NPC, NWIN, NPCP, REGSZ, NSW = _derive(cfg)

    per_core, shape, x_fp8 = _host_prep(
        inputs["x"], inputs["edge_index"], inputs["delta_agg"], cfg
    )
    WC, bC, idn, ones = _make_weight_arrays(
        inputs["W_mean"], inputs["b_mean"], inputs["W_ego"], inputs["b_ego"],
        inputs["W_nb"], inputs["b_nb"], cfg,
    )

    nc = _build_graph(
        cfg, shape, float(inputs["gate_weight"]), float(inputs["gate_bias"])
    )

    in_maps = []
    for ci in range(CORES):
        pc = per_core[ci]
        in_maps.append({
            "x": x_fp8,
            "xT": pc["xT"],
            "src_idx": pc["src_idx"],
            "S": pc["S"],
            "invdeg": pc["invdeg"],
            "delta": pc["delta"],
            "WC": WC,
            "bC": bC,
            "ident": idn,
            "ones": ones,
        })

    if sim:
        from concourse import bass_interp

        mcs = bass_interp.MultiCoreSim(nc, CORES)
        for ci in range(CORES):
            for k, v in in_maps[ci].items():
                mcs.cores[ci].tensor(k)[:] = v
        mcs.simulate(check_with_hw=False)
        outs = [
            np.array(mcs.cores[ci].mem_tensor("out"))
            .reshape(NPCP, F)[:NPC]
            .astype(np.float32)
            for ci in range(CORES)
        ]
        LAST_EXEC_NS = None
        return np.concatenate(outs, axis=0), None

    try:
        from bench_util import install_ntff_hook

        install_ntff_hook()
    except Exception:
        trace = False

    res = run_bass_kernel_spmd(
        nc, in_maps, core_ids=list(range(CORES)), trace=trace
    )
    LAST_RESULTS = res
    LAST_EXEC_NS = res.exec_time_ns
    outs = [
        res.results[ci]["out"].reshape(NPCP, F)[:NPC].astype(np.float32)
        for ci in range(CORES)
    ]
    return np.concatenate(outs, axis=0), res.exec_time_ns


def kernel(**inputs) -> np.ndarray:
    out, _ = run(inputs)
    return out.astype(np.float32)
